# revision 23
# baseline (speedup 1.0000x reference)
"""Trainium2 Bass kernel for a dense transformer block (B=2, T=2048, C=1024, nh=16, H=4096).

Strategy (8 NeuronCores, no device collectives):

  Launch 1 (head-parallel): cores 0-3 <- batch 0, cores 4-7 <- batch 1; each core
    handles 4 attention heads over the full sequence. x arrives pre-transposed
    (feature-major, bf16). LN1 statistics via ones-vector matmuls on TensorE;
    Q gets the full LN epilogue, K is centered only (its per-token LN scale is
    folded into the exp activation's per-partition scale operand, and the K
    bias cancels in softmax). Causal attention with 512-token query chunks,
    key-tile loop software-pipelined; exp on ScalarE; softmax denominator
    divides run inline per (head, chunk). Output: un-projected per-head y,
    feature-major bf16 [128, 2, 2048].

  Host: pure re-slicing (no arithmetic).

  Launch 2 (token-parallel, feature-major end-to-end, zero transposes): each
    core takes a 512-token slice: c_proj from the gathered y (+ residual + bias
    fused into one vector op per tile) -> LN2 (ones-matmul stats) -> c_fc ->
    gaussian activation (mu/sigma/fc_b folded into activation scale/bias;
    gamma/beta folded into fc2 weights/bias on host) -> c_fc2 (accumulated
    o-tile-outer, drains pipelined) -> fused residual add -> feature-major
    output slice (host transposes back). MLP runs in fp8-e4m3 DoubleRow
    (weights host-quantized with power-of-2 scales folded into the activation
    scale/bias and the output drain); everything else bf16 with f32 psum.
"""

import hashlib
import os
import shutil
from contextlib import ExitStack

import ml_dtypes
import numpy as np

import concourse.bass as bass
import concourse.tile as tile
from concourse import bacc, mybir
from concourse.bass_utils import run_bass_kernel_spmd

F32 = mybir.dt.float32
F8 = mybir.dt.float8e4
F8NP = ml_dtypes.float8_e4m3
F8SCALE_H2 = 16.0
F8SCALE_FCW = 2048.0
F8SCALE_ACT = 128.0
F8SCALE_FC2W = 8192.0
DR = mybir.MatmulPerfMode.DoubleRow


def q8w(w, scale):
    return np.clip(np.asarray(w, np.float32) * scale,
                   -240.0, 240.0).astype(F8NP)
F32R = mybir.dt.float32r
BF16 = mybir.dt.bfloat16
AF = mybir.ActivationFunctionType
ALU = mybir.AluOpType
BFNP = ml_dtypes.bfloat16

N_CORES = 8
T = 2048          # tokens per batch
C = 1024          # model dim
NH_LOC = 4        # heads per core (launch 1)
HS = 64           # head size
HID = 4096        # mlp hidden
TS = 512          # tokens per core (launch 2)

LAST_EXEC_NS = {}  # launch name -> exec_time_ns (filled when tracing enabled)

_CACHE_DIR = "/tmp/neff_cache"


def _install_compile_cache():
    import concourse.bass2jax as b2j

    if getattr(b2j, "_neff_cache_installed", False):
        return
    real = b2j.compile_bir_kernel

    def cached(bir_json, tmpdir, neff_name="file.neff"):
        os.makedirs(_CACHE_DIR, exist_ok=True)
        h = hashlib.sha256(bir_json).hexdigest()
        cpath = os.path.join(_CACHE_DIR, h + ".neff")
        out = os.path.join(tmpdir, neff_name)
        if os.path.exists(cpath):
            shutil.copyfile(cpath, out)
            return out
        res = real(bir_json, tmpdir, neff_name)
        shutil.copyfile(res, cpath)
        return res

    b2j.compile_bir_kernel = cached
    b2j._neff_cache_installed = True


# --------------------------------------------------------------------------
# Launch 1: LN1 + QKV + causal attention (4 heads) -> y feature-major
# --------------------------------------------------------------------------
def build_l1():
    nc = bacc.Bacc("TRN2", target_bir_lowering=False, debug=False,
                   num_devices=N_CORES)
    xT_d = nc.dram_tensor("xT", [4, 128, 8, 512], F8, kind="ExternalInput")
    wqk_d = nc.dram_tensor("wqkT", [128, 8, 512], F8, kind="ExternalInput")
    wv_d = nc.dram_tensor("wvT", [128, 8, 256], F8, kind="ExternalInput")
    vone8_d = nc.dram_tensor("vone8", [128, 1], F8, kind="ExternalInput")
    bq_d = nc.dram_tensor("bq", [128, 2], F32, kind="ExternalInput")
    wsqk_d = nc.dram_tensor("wsqk", [128, 4], F32, kind="ExternalInput")
    nwsv_d = nc.dram_tensor("nwsv", [1, 256], F32, kind="ExternalInput")
    mask_d = nc.dram_tensor("masks", [128, 4, 512], BF16, kind="ExternalInput")
    vones_d = nc.dram_tensor("vones", [128, 64], BF16, kind="ExternalInput")
    id_d = nc.dram_tensor("ident", [128, 128], F32, kind="ExternalInput")
    yT_d = nc.dram_tensor("yT", [128, 2, T], BF16, kind="ExternalOutput")

    n_tc = T // 512              # 4 attention query chunks

    with tile.TileContext(nc) as tc, ExitStack() as ctx:
        consts = ctx.enter_context(tc.tile_pool(name="consts", bufs=1))
        vones_sb = consts.tile([128, 64], BF16)
        nc.sync.dma_start(out=vones_sb[:], in_=vones_d[:])
        vone8_sb = consts.tile([128, 1], F8)
        nc.sync.dma_start(out=vone8_sb[:], in_=vone8_d[:])
        eps_sb = consts.tile([128, 1], F32)
        nc.vector.memset(eps_sb[:], 1e-5)
        bq_sb = consts.tile([128, 2], F32)
        nc.sync.dma_start(out=bq_sb[:], in_=bq_d[:])
        wsqk_sb = consts.tile([128, 4], F32)
        nc.sync.dma_start(out=wsqk_sb[:], in_=wsqk_d[:])
        ident = consts.tile([128, 128], F32)
        nc.sync.dma_start(out=ident[:], in_=id_d[:])
        nwsv_row = consts.tile([1, 256], F32)
        nc.sync.dma_start(out=nwsv_row[:], in_=nwsv_d[:])
        nwsv_b = consts.tile([128, 256], F32)
        nc.gpsimd.partition_broadcast(nwsv_b[:], nwsv_row[:])
        # per-key-tile LN scale / scale*mu columns (for exp scale + V epilogue)
        rall = consts.tile([128, 16], F32)
        rmuall = consts.tile([128, 16], F32)

        big2 = ctx.enter_context(tc.tile_pool(name="big2", bufs=1))
        qkT = big2.tile([128, 4, T], BF16)     # Q feats (tiles 0,1), K feats (2,3)
        v_sb = big2.tile([128, 16, NH_LOC, 128], BF16)  # col0=ones, 64:128=v
        yT = big2.tile([128, 2, T], BF16)

        wpool = ctx.enter_context(tc.tile_pool(name="wpool", bufs=1))
        wqk_sb = wpool.tile([128, 8, 512], F8)
        wv_sb = wpool.tile([128, 8, 256], F8)

        # ---- P1+P2 fused per 512-token chunk: raw QKV matmuls on un-normalized
        # xT; LayerNorm applied in the epilogues:
        #   q[f,t] = r[t]*raw[f,t] - r[t]*mu[t]*rowsum(Wq)[f] + bq[f]
        #   k[f,t] = raw[f,t] - mu[t]*rowsum(Wk)[f]   (r_key in exp scale,
        #                                              k bias cancels in softmax)
        with tc.tile_pool(name="p1", bufs=3) as p1, \
             tc.tile_pool(name="p1sl", bufs=3) as p1sl, \
             tc.tile_pool(name="p1b", bufs=2) as p1b, \
             tc.tile_pool(name="p1r", bufs=8) as p1r, \
             tc.tile_pool(name="p1psum", bufs=1, space="PSUM") as p1p, \
             tc.tile_pool(name="ptp", bufs=1, space="PSUM") as ptp, \
             tc.tile_pool(name="p2qk", bufs=3, space="PSUM") as p2qk, \
             tc.tile_pool(name="p2v", bufs=2, space="PSUM") as p2v:
            tpall = ptp.tile([128, 32], F32, tag="tp")
            slab0 = p1sl.tile([128, 8, 512], F8, tag="slab", name="slab0")
            for c in range(8):
                nc.sync.dma_start(out=slab0[:, c, :], in_=xT_d[0, :, c, :])
            for c in range(8):
                nc.sync.dma_start(out=wqk_sb[:, c, :], in_=wqk_d[:, c, :])
            for c in range(0, 8, 2):
                nc.sync.dma_start(out=wv_sb[:, c:c + 2, :],
                                  in_=wv_d[:, c:c + 2, :])
            for tch in range(n_tc):
                sl = slice(tch * 512, (tch + 1) * 512)
                if tch == 0:
                    slab = slab0
                else:
                    slab = p1sl.tile([128, 8, 512], F8, tag="slab",
                                     name=f"slab{tch}")
                    for c in range(0, 8, 2):
                        nc.sync.dma_start(out=slab[:, c:c + 2, :],
                                          in_=xT_d[tch, :, c:c + 2, :])
                ps_st = p1p.tile([1, 1024], F32, tag="st", name=f"st{tch}")
                for c in range(8):
                    sq = p1.tile([128, 512], BF16, tag="sq")
                    nc.scalar.activation(out=sq[:], in_=slab[:, c, :],
                                         func=AF.Square)
                    nc.tensor.matmul(ps_st[0:1, 0:512], vone8_sb[:],
                                     slab[:, c, :],
                                     start=(c == 0), stop=(c == 7),
                                     skip_group_check=True)
                    nc.tensor.matmul(ps_st[0:1, 512:1024], vones_sb[:, 0:1],
                                     sq[:],
                                     start=(c == 0), stop=(c == 7),
                                     skip_group_check=True)
                # raw Q/K projections (emitted before the row-math consumers
                # so the PE never waits on the LN chain)
                qk_ps = []
                for f in range(4):
                    ps = p2qk.tile([128, 512], F32, tag="qk",
                                   name=f"qk{tch}_{f}")
                    for g in range(4):
                        nc.tensor.matmul(
                            ps[:],
                            wqk_sb[:, 2 * g:2 * g + 2, f * 128:(f + 1) * 128],
                            slab[:, 2 * g:2 * g + 2, :],
                            start=(g == 0), stop=(g == 3), perf_mode=DR)
                    qk_ps.append(ps)
                # row math: mu, rstd, r*mu (rb broadcast issued as soon as
                # rrow exists so the Q epilogue unblocks early)
                mrow = p1r.tile([1, 512], F32, tag="row")
                nc.vector.tensor_scalar(out=mrow[:], in0=ps_st[0:1, 0:512],
                                        scalar1=1.0 / (16.0 * C),
                                        scalar2=None, op0=ALU.mult)
                msq = p1r.tile([1, 512], F32, tag="row")
                nc.vector.tensor_mul(msq[:], mrow[:], mrow[:])
                vrow = p1r.tile([1, 512], F32, tag="row")
                nc.vector.scalar_tensor_tensor(out=vrow[:],
                                               in0=ps_st[0:1, 512:1024],
                                               scalar=1.0 / (256.0 * C),
                                               in1=msq[:],
                                               op0=ALU.mult, op1=ALU.subtract)
                sdr = p1r.tile([1, 512], F32, tag="row")
                nc.scalar.activation(out=sdr[:], in_=vrow[:], func=AF.Sqrt,
                                     bias=eps_sb[0:1], scale=1.0)
                rrow = p1r.tile([1, 512], F32, tag="row")
                nc.vector.reciprocal_approx_fast(out=rrow[:], in_=sdr[:])
                rrow_q = p1r.tile([1, 512], F32, tag="rowq", bufs=2)
                nc.vector.tensor_scalar(out=rrow_q[:], in0=rrow[:],
                                        scalar1=1.0 / (16.0 * 16384.0),
                                        scalar2=None, op0=ALU.mult)
                rrow_s = p1r.tile([1, 512], F32, tag="rows", bufs=2)
                nc.vector.tensor_scalar(out=rrow_s[:], in0=rrow[:],
                                        scalar1=1.0 / (16.0 * 2048.0),
                                        scalar2=None, op0=ALU.mult)
                rb = p1b.tile([128, 512], F32, tag="rb")
                nc.gpsimd.partition_broadcast(rb[:], rrow_q[:])
                rmurow = p1r.tile([1, 512], F32, tag="row")
                nc.vector.tensor_mul(rmurow[:], rrow[:], mrow[:])
                rmu_b = p1b.tile([128, 512], F32, tag="rmu")
                nc.gpsimd.partition_broadcast(rmu_b[:], rmurow[:])
                mu_b = p1b.tile([128, 512], F32, tag="mub")
                nc.gpsimd.partition_broadcast(mu_b[:], mrow[:])
                # token-major per-partition columns of r and r*mu (for V + exp)
                for j in range(4):
                    idx = tch * 4 + j
                    nc.tensor.transpose(tpall[:, 2 * idx:2 * idx + 1],
                                        rrow_s[0:1, j * 128:(j + 1) * 128],
                                        ident[0:1, 0:1])
                    nc.vector.tensor_copy(rall[:, idx:idx + 1],
                                          tpall[:, 2 * idx:2 * idx + 1])
                    nc.tensor.transpose(tpall[:, 2 * idx + 1:2 * idx + 2],
                                        rmurow[0:1, j * 128:(j + 1) * 128],
                                        ident[0:1, 0:1])
                    nc.vector.tensor_copy(rmuall[:, idx:idx + 1],
                                          tpall[:, 2 * idx + 1:2 * idx + 2])
                # raw V (after transposes in PE order; epilogue needs rall cols)
                v_ps = []
                for tt4 in range(4):
                    ps = p2v.tile([128, 256], F32, tag="v",
                                  name=f"v{tch}_{tt4}")
                    for g in range(4):
                        nc.tensor.matmul(
                            ps[:],
                            slab[:, 2 * g:2 * g + 2,
                                 tt4 * 128:(tt4 + 1) * 128],
                            wv_sb[:, 2 * g:2 * g + 2, :],
                            start=(g == 0), stop=(g == 3), perf_mode=DR)
                    v_ps.append(ps)
                # Q epilogue (full LN)
                for f in range(2):
                    t1 = p1.tile([128, 512], F32, tag="t1")
                    nc.vector.tensor_mul(t1[:], qk_ps[f][:], rb[:])
                    t2 = p1.tile([128, 512], F32, tag="t2")
                    nc.vector.tensor_scalar(out=t2[:], in0=rmu_b[:],
                                            scalar1=wsqk_sb[:, f:f + 1],
                                            scalar2=bq_sb[:, f:f + 1],
                                            op0=ALU.mult, op1=ALU.subtract)
                    nc.vector.tensor_sub(qkT[:, f, sl], t1[:], t2[:])
                # K epilogue: k = raw + mu*(-ws_k), single fused op (gpsimd)
                for f in range(2, 4):
                    nc.vector.scalar_tensor_tensor(
                        out=qkT[:, f, sl], in0=mu_b[:],
                        scalar=wsqk_sb[:, f:f + 1], in1=qk_ps[f][:],
                        op0=ALU.mult, op1=ALU.add)
                # V epilogue
                for tt4 in range(4):
                    tt = tch * 4 + tt4
                    t1v = p1.tile([128, 256], F32, tag="t1v")
                    nc.vector.tensor_scalar(out=t1v[:], in0=v_ps[tt4][:],
                                            scalar1=rall[:, tt:tt + 1],
                                            scalar2=None, op0=ALU.mult)
                    nc.vector.scalar_tensor_tensor(
                        out=v_sb[:, tt, :, 64:128],
                        in0=nwsv_b[:].rearrange("p (h d) -> p h d", h=NH_LOC),
                        scalar=rmuall[:, tt:tt + 1],
                        in1=t1v[:].rearrange("p (h d) -> p h d", h=NH_LOC),
                        op0=ALU.mult, op1=ALU.add)
            nc.sync.dma_start(out=v_sb[:, :, :, 0:1], in_=vones_d[:])
            nc.vector.memset(v_sb[:, :, :, 1:64], 0.0)

        # ---- P3: attention, software-pipelined (QK for s+1 before AV of s);
        # exp scale carries the per-key LN scale; divides run inline ----
        with tc.tile_pool(name="p3consts", bufs=1) as p3c, \
             tc.tile_pool(name="p3a", bufs=6) as p3a, \
             tc.tile_pool(name="p3s", bufs=3) as p3s, \
             tc.tile_pool(name="p3ps2", bufs=2, space="PSUM") as p3ps2, \
             tc.tile_pool(name="p3ps1", bufs=2, space="PSUM") as p3ps1, \
             tc.tile_pool(name="p3py", bufs=1, space="PSUM") as p3py:
            mask_sb = p3c.tile([128, 4, 512], BF16)
            nc.sync.dma_start(out=mask_sb[:], in_=mask_d[:])
            for h in range(NH_LOC):
                po = (h % 2) * 64
                qf = h // 2
                kf = 2 + h // 2
                for pair in ((0, 1), (2, 3)):
                    smax = 4 * pair[1] + 4
                    pys = {}
                    for tcx in pair:
                        py_t = p3py.tile([128, 512], F32, tag=f"py{tcx % 2}",
                                         name=f"py{h}_{tcx}")
                        pys[tcx] = py_t

                    def emit_qk(s):
                        # diag tile: queries below 128*(s%4) in the first
                        # chunk are fully masked -- skip them everywhere
                        tiles = []
                        tcs = [tcx for tcx in pair if s <= 4 * tcx + 3]
                        if not tcs:
                            return tiles
                        wide = len(tcs) == 2
                        tot = 1024 if wide else 512
                        diag = (s // 4 == tcs[0])
                        off = 128 * (s % 4) if diag else 0
                        pool = p3ps2 if wide else p3ps1
                        pscore = pool.tile([128, tot], F32,
                                           tag="sc2" if wide else "sc1",
                                           name=f"sc{h}_{s}_{tcs[0]}")
                        for i, tcx in enumerate(tcs):
                            o0 = i * 512 + (off if i == 0 else 0)
                            nc.tensor.matmul(
                                pscore[:, o0:(i + 1) * 512],
                                qkT[po:po + 64, kf, s * 128:(s + 1) * 128],
                                qkT[po:po + 64, qf,
                                    tcx * 512 + (o0 - i * 512):(tcx + 1) * 512],
                                start=True, stop=True, skip_group_check=True)
                        at = p3a.tile([128, tot], BF16,
                                      tag="at2" if wide else "at1",
                                      name=f"at{h}_{s}_{tcs[0]}")
                        nc.scalar.activation(out=at[:, off:tot],
                                             in_=pscore[:, off:tot],
                                             func=AF.Exp,
                                             scale=rall[:, s:s + 1])
                        if diag:
                            m = s % 4
                            nc.vector.tensor_mul(at[:, off:512],
                                                 at[:, off:512],
                                                 mask_sb[:, m, off:512])
                        for i, tcx in enumerate(tcs):
                            o0 = i * 512 + (off if i == 0 else 0)
                            tiles.append((tcx, at[:, o0:(i + 1) * 512],
                                          o0 - i * 512))
                        return tiles

                    cur = emit_qk(0)
                    for s in range(smax):
                        nxt = emit_qk(s + 1) if s + 1 < smax else []
                        for tcx, atv, doff in cur:
                            nc.tensor.matmul(pys[tcx][:, doff:512],
                                             v_sb[:, s, h, :], atv,
                                             start=(s == 0),
                                             stop=(s == 4 * tcx + 3),
                                             skip_group_check=True)
                            if s == 4 * tcx + 3:
                                # inline softmax divide: row 64 = denominator
                                qsl = slice(tcx * 512, (tcx + 1) * 512)
                                rr = p3s.tile([1, 512], F32, tag="rr")
                                nc.vector.reciprocal_approx_fast(
                                    out=rr[:], in_=pys[tcx][0:1, :])
                                db = p3s.tile([64, 512], F32, tag="db")
                                nc.gpsimd.partition_broadcast(db[:], rr[:])
                                nc.vector.tensor_mul(
                                    yT[po:po + 64, h // 2, qsl],
                                    pys[tcx][64:128, :], db[:])
                                nc.sync.dma_start(
                                    out=yT_d[po:po + 64, h // 2, qsl],
                                    in_=yT[po:po + 64, h // 2, qsl])
                        cur = nxt
    nc.compile()
    return nc


# --------------------------------------------------------------------------
# Launch 2: c_proj + residual + LN2 + MLP + residual (feature-major)
# --------------------------------------------------------------------------
def build_l2(s_act: float):
    nc = bacc.Bacc("TRN2", target_bir_lowering=False, debug=False,
                   num_devices=N_CORES)
    yTs_d = nc.dram_tensor("yTs", [128, 8, 512], BF16, kind="ExternalInput")
    xTs_d = nc.dram_tensor("xTs", [128, 8, 512], BF16, kind="ExternalInput")
    pw_d = nc.dram_tensor("pwT", [128, 8, 1024], BF16, kind="ExternalInput")
    pb_d = nc.dram_tensor("pb", [128, 8], F32, kind="ExternalInput")
    fb2_d = nc.dram_tensor("fb2", [128, 8], F32, kind="ExternalInput")
    ab_d = nc.dram_tensor("abias", [128, 32], F32, kind="ExternalInput")
    fcw_d = nc.dram_tensor("fcwT", [8, 128, 8, 512], F8,
                           kind="ExternalInput")
    fc2w_d = nc.dram_tensor("fc2wT", [4, 128, 8, 1024], F8,
                            kind="ExternalInput")
    vone_d = nc.dram_tensor("vone", [128, 1], F32R, kind="ExternalInput")
    out_d = nc.dram_tensor("outT", [128, 8, 512], F32, kind="ExternalOutput")

    with tile.TileContext(nc) as tc, ExitStack() as ctx:
        consts = ctx.enter_context(tc.tile_pool(name="consts", bufs=1))
        eps_sb = consts.tile([128, 1], F32)
        nc.vector.memset(eps_sb[:], 1e-5 / (F8SCALE_H2 * F8SCALE_H2))
        vone_sb = consts.tile([128, 1], F32R)
        nc.sync.dma_start(out=vone_sb[:], in_=vone_d[:])
        pb_sb = consts.tile([128, 8], F32)
        nc.sync.dma_start(out=pb_sb[:], in_=pb_d[:])
        fb2_sb = consts.tile([128, 8], F32)
        nc.sync.dma_start(out=fb2_sb[:], in_=fb2_d[:])
        ab_sb = consts.tile([128, 32], F32)
        nc.sync.dma_start(out=ab_sb[:], in_=ab_d[:])
        lnact_sb = consts.tile([128, 1], F32)
        nc.vector.memset(lnact_sb[:], float(np.log(F8SCALE_ACT)))

        big = ctx.enter_context(tc.tile_pool(name="big", bufs=1))
        xTs = big.tile([128, 8, 512], BF16)
        x2T = big.tile([128, 8, 512], F32R)
        h2T = big.tile([128, 8, 512], F8)
        actT = big.tile([128, 32, 512], F8)
        fc2w_sb = big.tile([128, 32, 1024], F8)

        q2w = ctx.enter_context(tc.tile_pool(name="q2w", bufs=3))
        wts = []
        # ---- P1: c_proj + residual + pb (one fused drain per o-tile) ----
        with tc.tile_pool(name="q1", bufs=2) as q1, \
             tc.tile_pool(name="q1in", bufs=1) as q1in, \
             tc.tile_pool(name="q1r", bufs=6) as q1r, \
             tc.tile_pool(name="q1b", bufs=1) as q1b, \
             tc.tile_pool(name="q1psum", bufs=2, space="PSUM") as q1p, \
             tc.tile_pool(name="q1st", bufs=1, space="PSUM") as q1st:
            yTs = q1in.tile([128, 8, 512], BF16)
            for f in range(8):
                nc.sync.dma_start(out=yTs[:, f, :], in_=yTs_d[:, f, :])
            pw_sb = q1in.tile([128, 8, 1024], BF16)
            for f in range(8):
                nc.sync.dma_start(out=pw_sb[:, f, :], in_=pw_d[:, f, :])
            for f in range(0, 8, 2):
                nc.sync.dma_start(out=xTs[:, f:f + 2, :],
                                  in_=xTs_d[:, f:f + 2, :])
            for i in range(2):
                wt = q2w.tile([128, 8, 512], F8, tag="wt", name=f"wt{i}")
                nc.sync.dma_start(out=wt[:], in_=fcw_d[i])
                wts.append(wt)
            for hc in range(4):
                nc.sync.dma_start(
                    out=fc2w_sb[:].rearrange("p (c g) o -> p c g o", c=4)[:, hc],
                    in_=fc2w_d[hc].rearrange("p c (two g) -> p (c two) g", two=2)
                    if False else fc2w_d[hc])
            for o in range(8):
                ps = q1p.tile([128, 512], F32, tag="po", name=f"po{o}")
                for f in range(8):
                    nc.tensor.matmul(
                        ps[:], pw_sb[:, f, o * 128:(o + 1) * 128],
                        yTs[:, f, :], start=(f == 0), stop=(f == 7))
                nc.vector.scalar_tensor_tensor(
                    out=x2T[:, o, :], in0=ps[:], scalar=pb_sb[:, o:o + 1],
                    in1=xTs[:, o, :], op0=ALU.add, op1=ALU.add)
            # ---- LN2 stats (ones matmuls) + row math + h2T ----
            ps_st = q1st.tile([1, 1024], F32, tag="st")
            for o in range(8):
                sq = q1.tile([128, 512], F32R, tag="sq")
                nc.scalar.activation(out=sq[:], in_=x2T[:, o, :],
                                     func=AF.Square)
                nc.tensor.matmul(ps_st[0:1, 0:512], vone_sb[:], x2T[:, o, :],
                                 start=(o == 0), stop=(o == 7),
                                 skip_group_check=True)
                nc.tensor.matmul(ps_st[0:1, 512:1024], vone_sb[:], sq[:],
                                 start=(o == 0), stop=(o == 7),
                                 skip_group_check=True)
            mrow = q1r.tile([1, 512], F32, tag="row")
            nc.vector.tensor_scalar(out=mrow[:], in0=ps_st[0:1, 0:512],
                                    scalar1=1.0 / C,
                                    scalar2=None, op0=ALU.mult)
            msq = q1r.tile([1, 512], F32, tag="row")
            nc.vector.tensor_mul(msq[:], mrow[:], mrow[:])
            vrow = q1r.tile([1, 512], F32, tag="row")
            nc.vector.scalar_tensor_tensor(out=vrow[:],
                                           in0=ps_st[0:1, 512:1024],
                                           scalar=1.0 / C, in1=msq[:],
                                           op0=ALU.mult, op1=ALU.subtract)
            sdr = q1r.tile([1, 512], F32, tag="row")
            nc.scalar.activation(out=sdr[:], in_=vrow[:], func=AF.Sqrt,
                                 bias=eps_sb[0:1],
                                 scale=1.0 / (F8SCALE_H2 * F8SCALE_H2))
            rrow = q1r.tile([1, 512], F32, tag="row")
            nc.vector.reciprocal_approx_fast(out=rrow[:], in_=sdr[:])
            mu_b = q1b.tile([128, 512], F32, tag="mub")
            nc.gpsimd.partition_broadcast(mu_b[:], mrow[:])
            rb = q1b.tile([128, 512], F32, tag="rb")
            nc.gpsimd.partition_broadcast(rb[:], rrow[:])
            for o in range(8):
                tctr = q1.tile([128, 512], F32, tag="ctr")
                nc.vector.tensor_sub(tctr[:], x2T[:, o, :], mu_b[:])
                nc.vector.tensor_mul(h2T[:, o, :], tctr[:], rb[:])

        # ---- P2: c_fc + gaussian activation (feature-major hidden) ----
        with tc.tile_pool(name="q2psum", bufs=3, space="PSUM") as q2p:
            for hc in range(8):
                if hc + 2 < 8:
                    wt = q2w.tile([128, 8, 512], F8, tag="wt",
                                  name=f"wt{hc + 2}")
                    nc.sync.dma_start(out=wt[:], in_=fcw_d[hc + 2])
                    wts.append(wt)
                for ht in range(4):
                    pu = q2p.tile([128, 512], F32, tag="u",
                                  name=f"u{hc}_{ht}")
                    for g in range(4):
                        nc.tensor.matmul(
                            pu[:],
                            wts[hc][:, 2 * g:2 * g + 2,
                                    ht * 128:(ht + 1) * 128],
                            h2T[:, 2 * g:2 * g + 2, :],
                            start=(g == 0), stop=(g == 3), perf_mode=DR)
                    hi = hc * 4 + ht
                    usq = q2w.tile([128, 512], F32R, tag="usq", bufs=2)
                    nc.scalar.activation(out=usq[:], in_=pu[:], func=AF.Square,
                                         bias=ab_sb[:, hi:hi + 1],
                                         scale=s_act / (F8SCALE_H2 *
                                                        F8SCALE_FCW))
                    nc.scalar.activation(out=actT[:, hi, :], in_=usq[:],
                                         func=AF.Exp, scale=-1.0,
                                         bias=lnact_sb[:])

        # ---- P3: c_fc2, o-outer so drains+DMAs pipeline with the
        # next o-tile's accumulation ----
        with tc.tile_pool(name="q3o", bufs=2) as q3o, \
             tc.tile_pool(name="q3psum", bufs=2, space="PSUM") as q3p:
            for o in range(8):
                po = q3p.tile([128, 512], F32, tag="po2", name=f"po2_{o}")
                for g in range(16):
                    nc.tensor.matmul(
                        po[:],
                        fc2w_sb[:, 2 * g:2 * g + 2, o * 128:(o + 1) * 128],
                        actT[:, 2 * g:2 * g + 2, :],
                        start=(g == 0),
                        stop=(g == 15),
                        perf_mode=DR,
                        skip_group_check=True)
                osc = q3o.tile([128, 512], F32, tag="osc")
                nc.vector.tensor_scalar(
                    out=osc[:], in0=po[:],
                    scalar1=1.0 / (F8SCALE_ACT * F8SCALE_FC2W),
                    scalar2=fb2_sb[:, o:o + 1], op0=ALU.mult, op1=ALU.add)
                ot = q3o.tile([128, 512], F32, tag="ot")
                nc.vector.tensor_add(ot[:], osc[:], x2T[:, o, :])
                nc.sync.dma_start(out=out_d[:, o, :], in_=ot[:])
    nc.compile()
    return nc


# --------------------------------------------------------------------------
# Host-side orchestration
# --------------------------------------------------------------------------
_PROG_CACHE = {}


def _get_prog(key, builder, *args):
    if key not in _PROG_CACHE:
        _PROG_CACHE[key] = builder(*args)
    return _PROG_CACHE[key]


def _causal_masks4():
    s = np.arange(128)[:, None]
    t = np.arange(512)[None, :]
    ms = [((s + 128 * m) <= t).astype(BFNP) for m in range(4)]
    return np.ascontiguousarray(np.stack(ms, axis=1))  # [128, 4, 512]


def _perm(w, tiles, width):
    """[tiles*128, width] -> [128, tiles, width] (partition-major for DMA)."""
    return np.ascontiguousarray(w.reshape(tiles, 128, width).transpose(1, 0, 2))


def kernel(x, ln1_w, ln1_b, attn_w, attn_b, proj_w, proj_b,
           ln2_w, ln2_b, fc_w, fc_b, fc2_w, fc2_b,
           mu, sigma, gamma, beta, n_head):
    x = np.asarray(x, dtype=np.float32)
    attn_w = np.asarray(attn_w, dtype=np.float32)
    attn_b = np.asarray(attn_b, dtype=np.float32)
    proj_w = np.asarray(proj_w, dtype=np.float32)
    proj_b = np.asarray(proj_b, dtype=np.float32)
    fc_w = np.asarray(fc_w, dtype=np.float32)
    fc_b = np.asarray(fc_b, dtype=np.float32)
    fc2_w = np.asarray(fc2_w, dtype=np.float32)
    fc2_b = np.asarray(fc2_b, dtype=np.float32)
    ln1_w = np.asarray(ln1_w, dtype=np.float32)
    ln1_b = np.asarray(ln1_b, dtype=np.float32)
    ln2_w = np.asarray(ln2_w, dtype=np.float32)
    ln2_b = np.asarray(ln2_b, dtype=np.float32)
    mu = float(mu)
    sigma = float(sigma)
    gamma = float(gamma)
    beta = float(beta)
    n_head = int(n_head)

    B = x.shape[0]
    assert x.shape == (B, T, C) and B == 2 and n_head == 16

    _install_compile_cache()
    trace = bool(int(os.environ.get("BASS_KERNEL_TRACE", "0")))

    sig = abs(sigma) + 1e-8
    s_act = float(1.0 / (np.sqrt(2.0) * sig))

    # Fold LN affine params into the consuming projection weights (host-side).
    attn_w_eff = attn_w * ln1_w[None, :]
    attn_b_eff = attn_b + attn_w @ ln1_b
    fc_w_eff = fc_w * ln2_w[None, :]
    fc_b_eff = fc_b + fc_w @ ln2_b

    # ---- launch 1 ----
    nc1 = _get_prog(("l1",), build_l1)
    masks = _causal_masks4()
    vones = np.ones((128, 64), dtype=BFNP)
    ident = np.eye(128, dtype=np.float32)
    xb_T = [np.ascontiguousarray(q8w(x[b].T, 16.0)) for b in range(B)]
    in_maps1 = []
    for c in range(N_CORES):
        b, hg = c // 4, c % 4
        q_rows = attn_w_eff[hg * 256:(hg + 1) * 256] * 0.125
        k_rows = attn_w_eff[C + hg * 256:C + (hg + 1) * 256]
        v_rows = attn_w_eff[2 * C + hg * 256:2 * C + (hg + 1) * 256]
        wqk = np.concatenate([q_rows * 16384.0, k_rows * 2048.0], axis=0)
        wsums = np.concatenate([q_rows.sum(1),
                                -k_rows.sum(1) * (16.0 * 2048.0)])
        bq = attn_b_eff[hg * 256:(hg + 1) * 256] * 0.125
        m = {
            "xT": np.ascontiguousarray(
                _perm(xb_T[b], 8, T).reshape(128, 8, 4, 512)
                .transpose(2, 0, 1, 3)),
            "wqkT": _perm(q8w(wqk.T, 1.0), 8, 512),
            "wvT": _perm(q8w(v_rows.T, 2048.0), 8, 256),
            "bq": np.ascontiguousarray(bq.reshape(2, 128).T),
            "wsqk": np.ascontiguousarray(wsums.reshape(4, 128).T),
            "vone8": np.ones((128, 1), dtype=F8NP),
            "nwsv": np.ascontiguousarray(-v_rows.sum(axis=1)[None, :]),
            "masks": masks,
            "vones": vones,
            "ident": ident,
        }
        in_maps1.append(m)
    res1 = run_bass_kernel_spmd(nc1, in_maps1, list(range(N_CORES)), trace=trace)
    if res1.exec_time_ns is not None:
        LAST_EXEC_NS["l1"] = res1.exec_time_ns

    # reassemble y feature-major per batch: [1024, 2048] bf16
    yT_full = [np.empty((C, T), dtype=BFNP) for _ in range(B)]
    for c in range(N_CORES):
        b, hg = c // 4, c % 4
        yt = res1.results[c]["yT"]          # [128, 2, 2048] bf16
        for q in range(2):
            f0 = (hg * 4 + 2 * q) * 64
            yT_full[b][f0:f0 + 64, :] = yt[0:64, q, :]
            f1 = (hg * 4 + 2 * q + 1) * 64
            yT_full[b][f1:f1 + 64, :] = yt[64:128, q, :]

    # ---- launch 2 ----
    nc2 = _get_prog(("l2", s_act), build_l2, s_act)
    fc2w_eff = (gamma * fc2_w).T                        # [4096, 1024]
    fb2_eff = fc2_b + beta * fc2_w.sum(axis=1)
    abias = ((fc_b_eff - mu) * s_act).reshape(32, 128).T    # [128, 32]
    pwT = _perm(np.ascontiguousarray(proj_w.T.astype(BFNP)), 8, C)
    fcwT_p = _perm(q8w(fc_w_eff.T, F8SCALE_FCW), 8, HID)
    fcw_chunks = np.ascontiguousarray(
        fcwT_p.reshape(128, 8, 8, 512).transpose(2, 0, 1, 3))   # [8,128,8,512]
    fc2w_chunks = np.ascontiguousarray(
        q8w(fc2w_eff, F8SCALE_FC2W).reshape(4, 8, 128, 1024)
        .transpose(0, 2, 1, 3))                                 # [4,128,8,1024]
    bv_full = attn_b_eff[2 * C:3 * C]
    pb_eff = proj_b + proj_w @ bv_full       # V bias passes through softmax
    pb_t = np.ascontiguousarray(pb_eff.reshape(8, 128).T)
    fb2_t = np.ascontiguousarray(fb2_eff.reshape(8, 128).T)
    vone = np.ones((128, 1), dtype=np.float32)
    in_maps2 = []
    for c in range(N_CORES):
        b, slc = c // 4, c % 4
        t0 = slc * TS
        m = {
            "yTs": np.ascontiguousarray(
                yT_full[b][:, t0:t0 + TS].reshape(8, 128, TS)
                .transpose(1, 0, 2)),
            "xTs": np.ascontiguousarray(
                x[b].T[:, t0:t0 + TS].reshape(8, 128, TS)
                .transpose(1, 0, 2).astype(BFNP)),
            "pwT": pwT,
            "pb": pb_t,
            "fb2": fb2_t,
            "abias": np.ascontiguousarray(abias),
            "fcwT": fcw_chunks,
            "fc2wT": fc2w_chunks,
            "vone": vone,
        }
        in_maps2.append(m)
    res2 = run_bass_kernel_spmd(nc2, in_maps2, list(range(N_CORES)), trace=trace)
    if res2.exec_time_ns is not None:
        LAST_EXEC_NS["l2"] = res2.exec_time_ns

    out = np.empty((B, T, C), dtype=np.float32)
    for c in range(N_CORES):
        b, slc = c // 4, c % 4
        t0 = slc * TS
        ot = res2.results[c]["outT"]       # [128, 8, 512] f32
        out[b, t0:t0 + TS] = ot.transpose(2, 1, 0).reshape(TS, C)
    return out
